# revision 7
# baseline (speedup 1.0000x reference)
"""Multi-head self-attention (B=4, L=2048, D=512, H=4, Hd=128) on 8 TRN2 cores.

Sharding: core c handles batch b = c//2 and head-pair p = c%2 (heads 2p, 2p+1).
Each core computes a partial output y_part[b] = sum_{h in pair} ctx_h @ Wo_h.T;
host gathers: y[b] = y_part[core 2b] + y_part[core 2b+1] + bo.

Dataflow per core (matmuls bf16 inputs, fp32 PSUM accumulation):
  xT [512,2048] (host-pretransposed)  ->  QT,KT [hd,L] and V [L,hd] via PE
  scoresT [k,L_q] = KT_blk.T @ QT     (k-major: softmax along free dim never
  attnT = exp(scoresT/sqrt(hd))        needs a transpose anywhere)
  ctxT [hd,L_q] += V_blk.T @ attnT    (accumulate over k blocks)
  rowsum r = ones8.T @ sm8            (fp8 DoubleRow matmul: fold adds emit
                                       e4m3 pair tiles; 2x PE rate, and the
                                       positive-sum quantization error is
                                       ~0.2% -- far below bf16 path noise)
  rinv = 1/r on [1,L_q], broadcast to 128 partitions via gpsimd
  ctxT *= rinv ; y_blk [L_q,D] += ctxT_blk.T @ WoT_h (accumulate over heads)
Outproj is interleaved per query-window so PE never drains at the tail.
"""
import numpy as np
import ml_dtypes

B, L, D = 4, 2048, 512
H, HD = 4, 128
NCORES = 8
QW = 512          # query window (matmul N / PSUM bank)
NQC = L // QW     # 4 query windows
NKB = L // 128    # 16 key blocks
NDC = D // 128    # 4 contraction chunks for projections
SCALE = 1.0 / np.sqrt(HD)
# exp(s*x + EXPBIAS) = exp(s*x)/32: scales attn weights and rowsum equally
# (cancels in the normalize) while keeping the fp8 rowsum-fold tiles far
# from the float8e4 overflow threshold.
EXPBIAS = float(np.log(1.0 / 32.0))
NWARM = 24

_COMPILED = None


def _build():
    import concourse.bass as bass
    import concourse.mybir as mybir
    import concourse.tile as tile
    from concourse import bacc

    F32 = mybir.dt.float32
    BF16 = mybir.dt.bfloat16
    F8 = mybir.dt.float8e4
    AF = mybir.ActivationFunctionType
    DR = mybir.MatmulPerfMode.DoubleRow

    nc = bacc.Bacc("TRN2", target_bir_lowering=False, debug=False,
                   num_devices=NCORES)
    xT_d = nc.dram_tensor("xT", [D, L], BF16, kind="ExternalInput")
    wqT_d = nc.dram_tensor("wqT", [D, 256], BF16, kind="ExternalInput")
    wkT_d = nc.dram_tensor("wkT", [D, 256], BF16, kind="ExternalInput")
    wvT_d = nc.dram_tensor("wvT", [D, 256], BF16, kind="ExternalInput")
    woT_d = nc.dram_tensor("woT", [256, D], BF16, kind="ExternalInput")
    bq_d = nc.dram_tensor("bq", [128, 2], F32, kind="ExternalInput")
    bk_d = nc.dram_tensor("bk", [128, 2], F32, kind="ExternalInput")
    bv_d = nc.dram_tensor("bv", [1, 256], F32, kind="ExternalInput")
    y_d = nc.dram_tensor("y", [L, D], F32, kind="ExternalOutput")

    with tile.TileContext(nc) as tc:
        with (
            tc.tile_pool(name="singles", bufs=1) as singles,
            tc.tile_pool(name="pss", bufs=2, space="PSUM") as pss_pool,
            tc.tile_pool(name="psc", bufs=2, space="PSUM") as psc_pool,
            tc.tile_pool(name="psr", bufs=1, space="PSUM") as psr_pool,
            tc.tile_pool(name="psy", bufs=1, space="PSUM") as psy_pool,
            tc.tile_pool(name="attnp", bufs=12) as attnp,
            tc.tile_pool(name="smp", bufs=10) as smp,
            tc.tile_pool(name="recp", bufs=6) as recp,
            tc.tile_pool(name="ctup", bufs=2) as ctup,
            tc.tile_pool(name="yp", bufs=3) as yp,
        ):
            # ---- warm tiles first: PE warmup must not wait on input DMA ----
            warm_sb = singles.tile([128, 512], BF16)
            warmw_sb = singles.tile([128, 128], BF16)
            nc.gpsimd.memset(warmw_sb[:], 0.0)
            nc.gpsimd.memset(warm_sb[:], 0.0)

            # ---- load inputs, spread across 5 engine DMA rings ----
            xt_sb = singles.tile([128, NDC, L], BF16)
            wq_sb = singles.tile([128, NDC, 256], BF16)
            wk_sb = singles.tile([128, NDC, 256], BF16)
            wv_sb = singles.tile([128, NDC, 256], BF16)
            wo_sb = singles.tile([128, 2, D], BF16)
            xq = [nc.scalar, nc.gpsimd]
            for c in range(NDC):
                xq[c % 2].dma_start(xt_sb[:, c, :],
                                    xT_d[128 * c:128 * c + 128, :])
            for c in range(NDC):
                nc.sync.dma_start(wq_sb[:, c, :],
                                  wqT_d[128 * c:128 * c + 128, :])
            for c in range(NDC):
                nc.sync.dma_start(wk_sb[:, c, :],
                                  wkT_d[128 * c:128 * c + 128, :])
            for c in range(NDC):
                qeng = nc.scalar if c < 2 else nc.gpsimd
                qeng.dma_start(wv_sb[:, c, :], wvT_d[128 * c:128 * c + 128, :])
            for h in range(2):
                nc.sync.dma_start(wo_sb[:, h, :], woT_d[128 * h:128 * h + 128, :])
            bq_sb = singles.tile([128, 2], F32)
            bk_sb = singles.tile([128, 2], F32)
            nc.scalar.dma_start(bq_sb[:], bq_d[:])
            nc.gpsimd.dma_start(bk_sb[:], bk_d[:])
            bv1_sb = singles.tile([1, 256], F32)
            nc.gpsimd.dma_start(bv1_sb[:], bv_d[:])
            bv_sb = singles.tile([128, 256], F32)
            nc.gpsimd.partition_broadcast(bv_sb[:], bv1_sb[:])
            ones8_sb = singles.tile([128, 2, 16], F8)
            nc.vector.memset(ones8_sb[:], 1.0)
            ebias_sb = singles.tile([128, 1], F32)
            nc.vector.memset(ebias_sb[:], EXPBIAS)

            # PE warmup during the input-DMA window: dummy matmuls on memset
            # tiles lift HAM to 8/8 before the real burst arrives.
            for wi in range(NWARM):
                ps_w = psy_pool.tile([128, D], F32, name=f"ps_w{wi}", tag="psy")
                nc.tensor.matmul(ps_w[:], warmw_sb[:], warm_sb[:],
                                 start=True, stop=True)

            # ---- Q/K projections ----
            qt_sb = singles.tile([128, 2, L], BF16)   # QT per head [hd, L]
            kt_sb = singles.tile([128, 2, L], BF16)
            v_sb = singles.tile([128, NKB, 256], BF16)  # V [k-part, kblk, 2*hd]

            for h in range(2):
                for (w_sb, b_sb, o_sb) in ((wq_sb, bq_sb, qt_sb),
                                           (wk_sb, bk_sb, kt_sb)):
                    for qc in range(NQC):
                        win = slice(QW * qc, QW * qc + QW)
                        ps = pss_pool.tile([128, QW], F32,
                                           name=f"ps_p{h}{qc}", tag="pss")
                        for dc in range(NDC):
                            nc.tensor.matmul(
                                ps[:], w_sb[:, dc, 128 * h:128 * h + 128],
                                xt_sb[:, dc, win],
                                start=(dc == 0), stop=(dc == NDC - 1))
                        nc.vector.tensor_scalar_add(
                            o_sb[:, h, win], ps[:], b_sb[:, h:h + 1])

            def emit_vproj(lb):
                ps = pss_pool.tile([128, QW], F32, name=f"ps_v{lb}", tag="pss")
                for dc in range(NDC):
                    nc.tensor.matmul(
                        ps[:, 0:256], xt_sb[:, dc, 128 * lb:128 * lb + 128],
                        wv_sb[:, dc, :],
                        start=(dc == 0), stop=(dc == NDC - 1))
                nc.vector.tensor_add(v_sb[:, lb, :], ps[:, 0:256], bv_sb[:])

            # ---- attention: cross-iteration software pipeline ----
            # iteration t's PE stream: [ctx/rowsum of t-1 (dense, data ready)]
            # [finish t-1 off-PE] [outproj of an older window] [scores of t
            # (exp-paced stalls hidden behind the dense block)]
            ct_tiles = {}   # t -> [128, QW] bf16 normalized ctxT window

            def emit_scores_pair(t, qc, h, kk, sm8_prev):
                win = slice(QW * qc, QW * qc + QW)
                ps_s = pss_pool.tile([128, 1024], F32,
                                     name=f"ps_s{t}_{kk}", tag="pss")
                k0 = 256 * kk
                nc.tensor.matmul(ps_s[:, 0:512], kt_sb[:, h, k0:k0 + 128],
                                 qt_sb[:, h, win], start=True, stop=True)
                nc.tensor.matmul(ps_s[:, 512:1024],
                                 kt_sb[:, h, k0 + 128:k0 + 256],
                                 qt_sb[:, h, win], start=True, stop=True)
                at = attnp.tile([128, 1024], BF16, name=f"at{t}_{kk}",
                                tag="attn")
                nc.scalar.activation(at[:], ps_s[:], AF.Exp,
                                     bias=ebias_sb[:], scale=SCALE)
                # fold the two k-chunks (rowsum is k-agnostic) straight into
                # an fp8 pair tile: feeds a half-rate DoubleRow rowsum matmul
                if kk % 2 == 0:
                    sm8 = smp.tile([128, 2, 512], F8, name=f"sm{t}_{kk}",
                                   tag="sm")
                else:
                    sm8 = sm8_prev
                nc.vector.tensor_add(sm8[:, kk % 2, :],
                                     at[:, 0:512], at[:, 512:1024])
                return at, sm8

            def emit_ctx_pair(st, kk):
                t, h, at_tiles, ps_c, ps_r = st
                hs = slice(128 * h, 128 * h + 128)
                at, sm8 = at_tiles[kk]
                last = (kk == NKB // 2 - 1)
                nc.tensor.matmul(ps_c[:], v_sb[:, 2 * kk, hs], at[:, 0:512],
                                 start=(kk == 0), stop=False)
                nc.tensor.matmul(ps_c[:], v_sb[:, 2 * kk + 1, hs],
                                 at[:, 512:1024], start=False, stop=last)
                if kk % 2 == 1:
                    nc.tensor.matmul(ps_r[:], ones8_sb[:], sm8[:],
                                     start=(kk == 1), stop=last,
                                     perf_mode=DR)

            def finish_iter(st):
                t, h, at_tiles, ps_c, ps_r = st
                # free the ctx/rowsum PSUM banks, then normalize off-PE:
                # reciprocal on the [1,QW] row BEFORE broadcasting (the DVE
                # reciprocal is ~128x cheaper pre-broadcast), gpsimd SBUF
                # partition-broadcast instead of a DRAM bounce.
                ctu = ctup.tile([128, QW], F32, name=f"ctu{t}", tag="ctu")
                nc.vector.tensor_copy(ctu[:], ps_c[:])
                rrow = recp.tile([1, QW], F32, name=f"rr{t}", tag="rr")
                nc.vector.tensor_copy(rrow[:], ps_r[0:1, :])
                rinv = recp.tile([1, QW], F32, name=f"ri{t}", tag="ri")
                nc.vector.reciprocal(rinv[:], rrow[:])
                rec128 = recp.tile([128, QW], F32, name=f"rec128{t}",
                                   tag="rec128")
                nc.gpsimd.partition_broadcast(rec128[:], rinv[:])
                ct = singles.tile([128, QW], BF16, name=f"ct{t}")
                nc.vector.tensor_mul(ct[:], ctu[:], rec128[:])
                ct_tiles[t] = ct

            def emit_outproj1(qb, qsl, th0, th1):
                ps_y = psy_pool.tile([128, D], F32, name=f"ps_y{qb}",
                                     tag="psy")
                nc.tensor.matmul(ps_y[:], ct_tiles[th0][:, qsl],
                                 wo_sb[:, 0, :], start=True, stop=False)
                nc.tensor.matmul(ps_y[:], ct_tiles[th1][:, qsl],
                                 wo_sb[:, 1, :], start=False, stop=True)
                ysb = yp.tile([128, D], F32, name=f"ysb{qb}", tag="ysb")
                nc.vector.tensor_copy(ysb[:], ps_y[:])
                nc.sync.dma_start(y_d[128 * qb:128 * qb + 128, :], ysb[:])

            def emit_outproj2(qc):
                th0, th1 = 2 * qc, 2 * qc + 1
                for qq in range(NQC):
                    emit_outproj1(NQC * qc + qq,
                                  slice(128 * qq, 128 * qq + 128), th0, th1)

            prev = None
            for t in range(2 * NQC):
                qc, h = t // 2, t % 2
                ps_c = psc_pool.tile([128, QW], F32, name=f"ps_c{t}",
                                     tag="psc")
                ps_r = psr_pool.tile([16, QW], F32, name=f"ps_r{t}",
                                     tag="psr")
                at_tiles = []
                for kk in range(NKB // 2):
                    if prev is not None:
                        emit_ctx_pair(prev, kk)
                    elif kk < NKB // 4:
                        # iteration 0: V projection fills the exp-paced slack
                        emit_vproj(2 * kk)
                        emit_vproj(2 * kk + 1)
                    sm8_prev = at_tiles[-1][1] if kk % 2 == 1 else None
                    at_tiles.append(emit_scores_pair(t, qc, h, kk, sm8_prev))
                if prev is None:
                    for lb in range(NKB // 2, NKB):
                        emit_vproj(lb)
                else:
                    finish_iter(prev)
                    if t in (3, 5):
                        emit_outproj2((t - 3) // 2)
                prev = (t, h, at_tiles, ps_c, ps_r)
            # drain the pipeline: outproj(2) fills the exp-paced slips
            for kk in range(NKB // 2):
                emit_ctx_pair(prev, kk)
                if kk % 2 == 1:
                    qq = kk // 2
                    emit_outproj1(NQC * 2 + qq,
                                  slice(128 * qq, 128 * qq + 128), 4, 5)
            finish_iter(prev)
            emit_outproj2(3)

    nc.compile()
    return nc


def _get_compiled():
    global _COMPILED
    if _COMPILED is None:
        _COMPILED = _build()
    return _COMPILED


def make_in_maps(x, Wq, bq, Wk, bk, Wv, bv, Wo):
    bf16 = ml_dtypes.bfloat16
    xT = {b: np.ascontiguousarray(x[b].T).astype(bf16) for b in range(B)}
    WqT, WkT, WvT, WoT = (np.ascontiguousarray(W.T) for W in (Wq, Wk, Wv, Wo))
    in_maps = []
    for c in range(NCORES):
        b = c // 2
        p = c % 2
        hs = slice(256 * p, 256 * p + 256)
        in_maps.append({
            "xT": xT[b],
            "wqT": WqT[:, hs].astype(bf16),
            "wkT": WkT[:, hs].astype(bf16),
            "wvT": WvT[:, hs].astype(bf16),
            "woT": np.ascontiguousarray(WoT[hs, :]).astype(bf16),
            "bq": np.ascontiguousarray(bq[hs].reshape(2, 128).T),
            "bk": np.ascontiguousarray(bk[hs].reshape(2, 128).T),
            "bv": bv[hs].reshape(1, 256).copy(),
        })
    return in_maps


def kernel(x, Wq, bq, Wk, bk, Wv, bv, Wo, bo):
    from concourse.bass_utils import run_bass_kernel_spmd

    x = np.asarray(x, np.float32)
    Wq, Wk, Wv, Wo = (np.asarray(w, np.float32) for w in (Wq, Wk, Wv, Wo))
    bq, bk, bv, bo = (np.asarray(b, np.float32) for b in (bq, bk, bv, bo))

    in_maps = make_in_maps(x, Wq, bq, Wk, bk, Wv, bv, Wo)
    nc = _get_compiled()
    try:
        res = run_bass_kernel_spmd(nc, in_maps, list(range(NCORES)))
    except Exception:
        # one retry: transient device wedges usually clear on re-execution
        res = run_bass_kernel_spmd(nc, in_maps, list(range(NCORES)))
    y = np.empty((B, L, D), np.float32)
    for b in range(B):
        y[b] = res.results[2 * b]["y"] + res.results[2 * b + 1]["y"] + bo
    return y


# revision 25
# speedup vs baseline: 1.0167x; 1.0167x over previous
"""Multi-head self-attention (B=4, L=2048, D=512, H=4, Hd=128) on 8 TRN2 cores.

Sharding: core c handles batch b = c//2 and head-pair p = c%2 (heads 2p, 2p+1).
Each core computes a partial output y_part[b] = sum_{h in pair} ctx_h @ Wo_h.T
with an UNBIASED V projection; the V bias contributes exactly
(sum_h bv_h @ Wo_h.T) to every query after softmax-normalization, so the host
folds it into the gather: y[b] = y_part[2b] + y_part[2b+1] + bo + Wo @ bv.

Dataflow per core (all matmuls bf16 inputs, fp32 PSUM accumulation):
  xT [512,2048] (host-pretransposed)  ->  QT,KT [hd,L] and V [L,hd] via PE
  scoresT [k,L_q] = KT_blk.T @ QT     (k-major: softmax along free dim never
  attnT = exp(scoresT/sqrt(hd))        needs a transpose anywhere)
  ctxT [hd,L_q] += V_blk.T @ attnT    (accumulate over k blocks)
  r = ones.T @ fold(attnT)            (partition-dim reduce via matmul)
  1/r on a [2,L_q] pair tile (one DVE reciprocal per WINDOW, not per head:
  DVE cost is per-lane-serial so batching rows is free), gpsimd
  partition_broadcast (no DRAM bounce), ctxT *= 1/r
  y_blk [L_q,D] += ctxT_blk.T @ WoT_h (accumulate over the 2 heads)
Outproj is spread 2 query-tiles per iteration so PE never bursts or drains.
"""
import numpy as np
import ml_dtypes

B, L, D = 4, 2048, 512
H, HD = 4, 128
NCORES = 8
QW = 512          # query window (matmul N / PSUM bank)
NQC = L // QW     # 4 query windows
NKB = L // 128    # 16 key blocks
NDC = D // 128    # 4 contraction chunks for projections
SCALE = 1.0 / np.sqrt(HD)
NWARM = 28

_COMPILED = None


def _build():
    import concourse.bass as bass
    import concourse.mybir as mybir
    import concourse.tile as tile
    from concourse import bacc

    F32 = mybir.dt.float32
    BF16 = mybir.dt.bfloat16
    AF = mybir.ActivationFunctionType

    nc = bacc.Bacc("TRN2", target_bir_lowering=False, debug=False,
                   num_devices=NCORES)
    xT_d = nc.dram_tensor("xT", [D, L], BF16, kind="ExternalInput")
    wqT_d = nc.dram_tensor("wqT", [D, 256], BF16, kind="ExternalInput")
    wkT_d = nc.dram_tensor("wkT", [D, 256], BF16, kind="ExternalInput")
    wvT_d = nc.dram_tensor("wvT", [D, 256], BF16, kind="ExternalInput")
    woT_d = nc.dram_tensor("woT", [256, D], BF16, kind="ExternalInput")
    bq_d = nc.dram_tensor("bq", [128, 2], F32, kind="ExternalInput")
    bk_d = nc.dram_tensor("bk", [128, 2], F32, kind="ExternalInput")
    y_d = nc.dram_tensor("y", [L, D], F32, kind="ExternalOutput")

    with tile.TileContext(nc) as tc:
        with (
            tc.tile_pool(name="singles", bufs=1) as singles,
            tc.tile_pool(name="pss", bufs=2, space="PSUM") as pss_pool,
            tc.tile_pool(name="psc", bufs=2, space="PSUM") as psc_pool,
            tc.tile_pool(name="psr", bufs=1, space="PSUM") as psr_pool,
            tc.tile_pool(name="psy", bufs=1, space="PSUM") as psy_pool,
            tc.tile_pool(name="attnp", bufs=12) as attnp,
            tc.tile_pool(name="smp", bufs=18) as smp,
            tc.tile_pool(name="recp", bufs=4) as recp,
            tc.tile_pool(name="ctup", bufs=2) as ctup,
            tc.tile_pool(name="yp", bufs=3) as yp,
        ):
            # ---- warm tiles first: PE warmup must not wait on input DMA ----
            warm_sb = singles.tile([128, 512], BF16)
            warmw_sb = singles.tile([128, 128], BF16)
            nc.gpsimd.memset(warmw_sb[:], 0.0)
            nc.gpsimd.memset(warm_sb[:], 0.0)

            # ---- load inputs: xT on scalar+gpsimd rings, weights on sync ----
            xt_sb = singles.tile([128, NDC, L], BF16)
            wq_sb = singles.tile([128, NDC, 256], BF16)
            wk_sb = singles.tile([128, NDC, 256], BF16)
            wv_sb = singles.tile([128, NDC, 256], BF16)
            wo_sb = singles.tile([128, 2, D], BF16)
            xq = [nc.scalar, nc.gpsimd]
            for c in range(NDC):
                xq[c % 2].dma_start(xt_sb[:, c, :],
                                    xT_d[128 * c:128 * c + 128, :])
            for c in range(NDC):
                nc.sync.dma_start(wq_sb[:, c, :],
                                  wqT_d[128 * c:128 * c + 128, :])
            for c in range(NDC):
                nc.sync.dma_start(wk_sb[:, c, :],
                                  wkT_d[128 * c:128 * c + 128, :])
            for c in range(NDC):
                qeng = nc.scalar if c < 2 else nc.gpsimd
                qeng.dma_start(wv_sb[:, c, :], wvT_d[128 * c:128 * c + 128, :])
            for h in range(2):
                nc.sync.dma_start(wo_sb[:, h, :], woT_d[128 * h:128 * h + 128, :])
            bq_sb = singles.tile([128, 2], F32)
            bk_sb = singles.tile([128, 2], F32)
            nc.scalar.dma_start(bq_sb[:], bq_d[:])
            nc.gpsimd.dma_start(bk_sb[:], bk_d[:])
            ones_sb = singles.tile([128, 1], BF16)
            nc.vector.memset(ones_sb[:], 1.0)
            onesf_sb = singles.tile([33, 128], F32)
            nc.vector.memset(onesf_sb[:], 1.0)

            # PE warmup during the input-DMA window: dummy matmuls on memset
            # tiles lift HAM to 8/8 before the real burst arrives.
            for wi in range(NWARM):
                ps_w = psy_pool.tile([128, D], F32, name=f"ps_w{wi}", tag="psy")
                nc.tensor.matmul(ps_w[:], warmw_sb[:], warm_sb[:],
                                 start=True, stop=True)

            # ---- Q/K projections (bias adds on ACT: it idles until exps) ----
            qt_sb = singles.tile([128, 2, L], BF16)   # QT per head [hd, L]
            kt_sb = singles.tile([128, 2, L], BF16)
            v_sb = singles.tile([128, NKB, 256], BF16)  # V [k-part, kblk, 2*hd]

            for h in range(2):
                for (w_sb, b_sb, o_sb) in ((wq_sb, bq_sb, qt_sb),
                                           (wk_sb, bk_sb, kt_sb)):
                    for qc in range(NQC):
                        win = slice(QW * qc, QW * qc + QW)
                        ps = pss_pool.tile([128, QW], F32,
                                           name=f"ps_p{h}{qc}", tag="pss")
                        for dc in range(NDC):
                            nc.tensor.matmul(
                                ps[:], w_sb[:, dc, 128 * h:128 * h + 128],
                                xt_sb[:, dc, win],
                                start=(dc == 0), stop=(dc == NDC - 1))
                        nc.scalar.activation(o_sb[:, h, win], ps[:],
                                             AF.Identity,
                                             bias=b_sb[:, h:h + 1], scale=1.0)

            def emit_vproj(lb):
                ps = pss_pool.tile([128, QW], F32, name=f"ps_v{lb}", tag="pss")
                for dc in range(NDC):
                    nc.tensor.matmul(
                        ps[:, 0:256], xt_sb[:, dc, 128 * lb:128 * lb + 128],
                        wv_sb[:, dc, :],
                        start=(dc == 0), stop=(dc == NDC - 1))
                nc.vector.tensor_copy(v_sb[:, lb, :], ps[:, 0:256])

            # ---- attention: cross-iteration software pipeline ----
            # iteration t's PE stream interleaves, per kk-pair slot:
            # [ctx pair of t-1 (dense, data ready)] [rowsum of t-1] [2 spread
            # outproj tiles of an older window at kk=2,5] [scores pair of t
            # (exp-paced, hidden behind the dense block)]
            ct_tiles = {}    # t -> [128, QW] bf16 normalized ctxT window
            ctu_tiles = {}   # t -> [128, QW] f32 unnormalized ctxT
            pr_tiles = {}    # pair m -> [2, QW] f32 rowsum psum (row = t%2)
            pcs = {}

            def emit_scores_pair(t, qc, h, kk):
                win = slice(QW * qc, QW * qc + QW)
                ps_s = pss_pool.tile([128, 1024], F32,
                                     name=f"ps_s{t}_{kk}", tag="pss")
                k0 = 256 * kk
                nc.tensor.matmul(ps_s[:, 0:512], kt_sb[:, h, k0:k0 + 128],
                                 qt_sb[:, h, win], start=True, stop=True)
                nc.tensor.matmul(ps_s[:, 512:1024],
                                 kt_sb[:, h, k0 + 128:k0 + 256],
                                 qt_sb[:, h, win], start=True, stop=True)
                at = attnp.tile([128, 1024], BF16, name=f"at{t}_{kk}",
                                tag="attn")
                nc.scalar.activation(at[:], ps_s[:], AF.Exp, scale=SCALE)
                # fold the two k-chunks: rowsum is k-agnostic, so one DVE add
                # halves the rowsum matmul count
                sm = smp.tile([128, 512], BF16, name=f"sm{t}_{kk}", tag="sm")
                nc.vector.tensor_add(sm[:], at[:, 0:512], at[:, 512:1024])
                return at, sm

            def emit_ctx_pair(st, kk):
                t, h, at_tiles, ps_c, ps_r = st
                hs = slice(128 * h, 128 * h + 128)
                at, sm = at_tiles[kk]
                last = (kk == NKB // 2 - 1)
                nc.tensor.matmul(ps_c[:], v_sb[:, 2 * kk, hs], at[:, 0:512],
                                 start=(kk == 0), stop=False)
                nc.tensor.matmul(ps_c[:], v_sb[:, 2 * kk + 1, hs],
                                 at[:, 512:1024], start=False, stop=last)

            def emit_rowsums(st):
                # hoisted to the START of the next iteration: the folds are
                # all complete, so this 2.4us batch runs dense AND the pair's
                # reciprocal chain gets a full iteration of slack before the
                # spread outproj tiles consume the cts.
                t, h, at_tiles, ps_c, ps_r = st
                row = 32 * (t % 2)
                for kk in range(NKB // 2):
                    nc.tensor.matmul(ps_r[row:row + 1, :], ones_sb[:],
                                     at_tiles[kk][1][:], start=(kk == 0),
                                     stop=(kk == NKB // 2 - 1))

            def emit_ctu(t, on_act=False):
                # free the ctx PSUM bank; normalization happens per-pair later
                ctu = ctup.tile([128, QW], F32, name=f"ctu{t}", tag="ctu")
                if on_act:
                    nc.scalar.activation(ctu[:], pcs[t][:], AF.Copy)
                else:
                    nc.vector.tensor_copy(ctu[:], pcs[t][:])
                ctu_tiles[t] = ctu

            rec_tiles = {}

            def emit_ct_mul(t):
                ct = singles.tile([128, QW], BF16, name=f"ct{t}")
                nc.vector.tensor_mul(ct[:], ctu_tiles[t][:], rec_tiles[t][:])
                ct_tiles[t] = ct

            def finish_pair_recip(m, bcast=True):
                # one [2,QW] copy + ONE reciprocal for both heads of window m
                # (DVE cost is per-lane-serial: a [2,512] op costs the same as
                # [1,512], halving the per-iteration reciprocal burden)
                # rows 0/32: SBUF APs must start at partition 0/32/64/96,
                # and row 32 can feed a matmul rhs / partition_broadcast
                # directly; rows 1..31 are memset so the reciprocal of the
                # unused lanes stays finite
                rr = recp.tile([33, QW], F32, name=f"rr{m}", tag="rr")
                nc.vector.memset(rr[:], 1.0)
                nc.vector.tensor_copy(rr[0:1, :], pr_tiles[m][0:1, :])
                nc.vector.tensor_copy(rr[32:33, :], pr_tiles[m][32:33, :])
                ri = recp.tile([33, QW], F32, name=f"ri{m}", tag="ri")
                nc.vector.reciprocal(ri[:], rr[:])
                if not bcast:
                    return ri
                # the gpsimd broadcast ucode only reads a partition-0 input
                # (a base-32 AP returns garbage on HW), so extract row 32
                rx = recp.tile([1, QW], F32, name=f"rx{m}", tag="rx")
                nc.vector.tensor_copy(rx[:], ri[32:33, :])
                for j, src in ((0, ri[0:1, :]), (1, rx[:])):
                    t = 2 * m + j
                    rec = recp.tile([128, QW], F32, name=f"rec{t}", tag="rec")
                    nc.gpsimd.partition_broadcast(rec[:], src)
                    rec_tiles[t] = rec
                # head 2m's unnormalized ctx already exists; head 2m+1's is
                # still accumulating, its mul follows this iteration's ctu.
                emit_ct_mul(2 * m)

            def emit_outproj1(qb, pool, ysb_act=False):
                qc, qq = qb // NQC, qb % NQC
                qsl = slice(128 * qq, 128 * qq + 128)
                ps_y = pool.tile([128, D], F32, name=f"ps_y{qb}",
                                 tag="psy" if pool is psy_pool else "psr")
                nc.tensor.matmul(ps_y[:], ct_tiles[2 * qc][:, qsl],
                                 wo_sb[:, 0, :], start=True, stop=False)
                nc.tensor.matmul(ps_y[:], ct_tiles[2 * qc + 1][:, qsl],
                                 wo_sb[:, 1, :], start=False, stop=True)
                ysb = yp.tile([128, D], F32, name=f"ysb{qb}", tag="ysb")
                if ysb_act:
                    # drain-phase copies ride the ACT engine (exps are done)
                    nc.scalar.activation(ysb[:], ps_y[:], AF.Copy)
                else:
                    nc.vector.tensor_copy(ysb[:], ps_y[:])
                nc.sync.dma_start(y_d[128 * qb:128 * qb + 128, :], ysb[:])

            # outproj spread: window qc's 4 query-tiles are emitted at kk=2/5
            # of iterations 2qc+3 and 2qc+4 (its cts exist after finish_pair
            # at the end of iteration 2qc+2); windows 2/3 drain at the end.
            opj_sched = {(3, 2): 0, (3, 5): 1, (4, 2): 2, (4, 5): 3,
                         (5, 2): 4, (5, 5): 5, (6, 2): 6, (6, 5): 7,
                         (7, 2): 8, (7, 5): 9}

            prev = None
            for t in range(2 * NQC):
                qc, h = t // 2, t % 2
                ps_c = psc_pool.tile([128, QW], F32, name=f"ps_c{t}",
                                     tag="psc")
                pcs[t] = ps_c
                if t % 2 == 0:
                    # rows 0/32: matmul outputs must start at partition 0/32/64
                    pr_tiles[t // 2] = psr_pool.tile([64, QW], F32,
                                                     name=f"ps_r{t}",
                                                     tag="psr")
                ps_r = pr_tiles[t // 2]
                at_tiles = []
                if prev is not None:
                    # scores(t,0) leads so the ACT exp stream starts before
                    # the dense rowsum+ctx block; otherwise the 8 exps
                    # overhang the iteration and stall the next one's scores
                    at_tiles.append(emit_scores_pair(t, qc, h, 0))
                    emit_rowsums(prev)
                    if t >= 2 and t % 2 == 0:
                        finish_pair_recip(t // 2 - 1)
                    for kk in range(NKB // 2):
                        emit_ctx_pair(prev, kk)
                        if prev[0] == 0 and kk < 2:
                            emit_vproj(14 + kk)
                        if (t, kk) in opj_sched:
                            emit_outproj1(opj_sched[(t, kk)], psy_pool)
                        if kk < NKB // 2 - 1:
                            at_tiles.append(
                                emit_scores_pair(t, qc, h, kk + 1))
                    emit_ctu(prev[0])
                    if t >= 2 and t % 2 == 0:
                        emit_ct_mul(t - 1)
                else:
                    for kk in range(NKB // 2):
                        if kk < NKB // 4:
                            # iteration 0: V proj fills the exp-paced slack
                            emit_vproj(2 * kk)
                            emit_vproj(2 * kk + 1)
                        at_tiles.append(emit_scores_pair(t, qc, h, kk))
                    for lb in range(NKB // 2, NKB - 2):
                        emit_vproj(lb)
                prev = (t, h, at_tiles, ps_c, ps_r)
            # drain: rowsums of t=7 first (their folds are already done), so
            # the last pair's reciprocal runs during the drain ctx matmuls;
            # its broadcast uses idle-PE fp32 rank-1 matmuls and the PSUM
            # copies ride the idle ACT engine, keeping DVE off the tail path
            emit_rowsums(prev)
            ri3 = finish_pair_recip(3, bcast=False)
            for kk in range(NKB // 2):
                emit_ctx_pair(prev, kk)
                if kk == 1:
                    emit_outproj1(10, psy_pool, ysb_act=True)
                if kk == 3:
                    emit_outproj1(11, psy_pool, ysb_act=True)
            emit_ctu(7, on_act=True)
            for j in (0, 1):
                ps_b = pss_pool.tile([128, QW], F32, name=f"ps_b{j}",
                                     tag="pss")
                nc.tensor.matmul(ps_b[:], onesf_sb[32 * j:32 * j + 1, :],
                                 ri3[32 * j:32 * j + 1, :],
                                 start=True, stop=True)
                rec_tiles[6 + j] = ps_b
            emit_ct_mul(6)
            emit_ct_mul(7)
            for qb in range(12, 16):
                emit_outproj1(qb, psy_pool if qb % 2 == 0 else psr_pool,
                              ysb_act=True)

    nc.compile()
    return nc


def _get_compiled():
    global _COMPILED
    if _COMPILED is None:
        _COMPILED = _build()
    return _COMPILED


def make_in_maps(x, Wq, bq, Wk, bk, Wv, bv, Wo):
    bf16 = ml_dtypes.bfloat16
    xT = {b: np.ascontiguousarray(x[b].T).astype(bf16) for b in range(B)}
    WqT, WkT, WvT, WoT = (np.ascontiguousarray(W.T) for W in (Wq, Wk, Wv, Wo))
    in_maps = []
    for c in range(NCORES):
        b = c // 2
        p = c % 2
        hs = slice(256 * p, 256 * p + 256)
        in_maps.append({
            "xT": xT[b],
            "wqT": WqT[:, hs].astype(bf16),
            "wkT": WkT[:, hs].astype(bf16),
            "wvT": WvT[:, hs].astype(bf16),
            "woT": np.ascontiguousarray(WoT[hs, :]).astype(bf16),
            "bq": np.ascontiguousarray(bq[hs].reshape(2, 128).T),
            "bk": np.ascontiguousarray(bk[hs].reshape(2, 128).T),
        })
    return in_maps


def kernel(x, Wq, bq, Wk, bk, Wv, bv, Wo, bo):
    from concourse.bass_utils import run_bass_kernel_spmd

    x = np.asarray(x, np.float32)
    Wq, Wk, Wv, Wo = (np.asarray(w, np.float32) for w in (Wq, Wk, Wv, Wo))
    bq, bk, bv, bo = (np.asarray(b, np.float32) for b in (bq, bk, bv, bo))

    in_maps = make_in_maps(x, Wq, bq, Wk, bk, Wv, bv, Wo)
    nc = _get_compiled()
    try:
        res = run_bass_kernel_spmd(nc, in_maps, list(range(NCORES)))
    except Exception:
        # one retry: transient device wedges usually clear on re-execution
        res = run_bass_kernel_spmd(nc, in_maps, list(range(NCORES)))
    # V-bias folds to a constant row after softmax: + Wo @ bv (all heads)
    yconst = bo + Wo @ bv
    y = np.empty((B, L, D), np.float32)
    for b in range(B):
        y[b] = res.results[2 * b]["y"] + res.results[2 * b + 1]["y"] + yconst
    return y
